# revision 3
# baseline (speedup 1.0000x reference)
"""Trainium2 Bass kernel for nn_AllAttLayer (cross-batch attention gating layer).

Reference computation (B=8, C=512, H=W=32, HW=1024):
    xf = x as [B, HW, C]
    q = xf @ Wq.T + bq ; k = xf @ Wk.T + bk
    scores = q.flat @ k.flat.T                  # [B*HW, B*HW]
    xw = max over each image's keys, mean over images   # [B*HW]
    xw = softmax(xw * C**-0.5 per image)        # [B, HW]
    out = (x * xw) @ W6.T + b6  (1x1 conv)      # == (W6 @ x) * xw

Sharding: core b owns image b (its 1024 queries). Keys are computed
locally per shard and AllGathered (2 chunks of 512 keys for overlap).
Everything is kept c-major ([C, HW]: channel on partitions, pixel on
free dim) so PE matmuls need no transposes:
    qT = Wq @ x_b   (lhsT = Wq.T tile, rhs = x tile)
    scores[q, key] : lhsT = qT tile [c,128q], rhs = kT tile [c,512key]
The per-query gating weight commutes with the final 1x1 conv, so we
compute y = W6 @ x_b + b6 early and multiply by the broadcast softmax
row at the end.
"""

import sys
import numpy as np

for _p in ("/opt/trn_rl_repo",):
    if _p not in sys.path:
        sys.path.insert(0, _p)

B, C, H, W = 8, 512, 32, 32
HW = H * W              # 1024 pixels per image
NCORES = 8
CB = C // 128           # 4 channel blocks
QB = HW // 128          # 8 query blocks per core
KH = 2                  # key halves (AllGather chunks of 512 keys)
SCALE = 1.0 / float(np.sqrt(C))

# "f32r": fp32 data, PE transpose-mode matmuls (1 cyc/row at N=512).
#         NOTE: walrus requires f32r matmul inputs to be produced by a
#         rounding compute instruction (not DMA) - needs extra passes.
# "bf16": bf16 matmul inputs (FWL weight loads, half AG traffic).
# "f32":  plain fp32 matmuls (4 cyc/row) - correctness fallback.
MM_MODE = "bf16"


def build_kernel(mode=MM_MODE):
    from concourse import bass, bacc, tile, mybir

    f32 = mybir.dt.float32
    if mode == "bf16":
        mmdt = mybir.dt.bfloat16
    else:
        mmdt = f32

    def mm_ap(ap):
        """AP view handed to the TensorEngine."""
        if mode == "f32r":
            return ap.bitcast(mybir.dt.float32r)
        return ap

    nc = bacc.Bacc("TRN2", target_bir_lowering=False, debug=False,
                   num_devices=NCORES)

    x_in = nc.dram_tensor("x", [C, HW], f32, kind="ExternalInput").ap()
    wqt_in = nc.dram_tensor("wqt", [C, C], f32, kind="ExternalInput").ap()
    wkt_in = nc.dram_tensor("wkt", [C, C], f32, kind="ExternalInput").ap()
    w6t_in = nc.dram_tensor("w6t", [C, C], f32, kind="ExternalInput").ap()
    bq_in = nc.dram_tensor("bq", [C, 1], f32, kind="ExternalInput").ap()
    bk_in = nc.dram_tensor("bk", [C, 1], f32, kind="ExternalInput").ap()
    b6_in = nc.dram_tensor("b6", [C, 1], f32, kind="ExternalInput").ap()
    out_ext = nc.dram_tensor("out", [C, HW], f32, kind="ExternalOutput").ap()

    AF = mybir.ActivationFunctionType
    ALU = mybir.AluOpType
    AX = mybir.AxisListType

    with tile.TileContext(nc) as tc:
        with tc.tile_pool(name="consts", bufs=1) as consts, \
             tc.tile_pool(name="wpool", bufs=1) as wpool, \
             tc.tile_pool(name="xpool", bufs=1) as xpool, \
             tc.tile_pool(name="qpool", bufs=1) as qpool, \
             tc.tile_pool(name="kepool", bufs=2) as kepool, \
             tc.tile_pool(name="kinpool", bufs=2) as kinpool, \
             tc.tile_pool(name="redpool", bufs=1) as redpool, \
             tc.tile_pool(name="outpool", bufs=2) as outpool, \
             tc.tile_pool(name="dram", bufs=1, space="DRAM") as dram, \
             tc.tile_pool(name="ps_s", bufs=5, space="PSUM") as ps_s, \
             tc.tile_pool(name="ps_m", bufs=2, space="PSUM") as ps_m:

            # ---- constants / biases ----
            ones_col = consts.tile([128, 1], f32, tag="ones_col")
            nc.vector.memset(ones_col[:], 1.0)
            ones_row = consts.tile([1, 128], f32, tag="ones_row")
            nc.vector.memset(ones_row[:], 1.0)

            bias_sb = {}
            for nm, src in (("bq", bq_in), ("bk", bk_in), ("b6", b6_in)):
                t = consts.tile([128, CB], f32, tag=f"{nm}_sb", name=f"{nm}_sb")
                for co in range(CB):
                    nc.sync.dma_start(out=t[:, co:co + 1],
                                      in_=src[co * 128:(co + 1) * 128, :])
                bias_sb[nm] = t

            # ---- weights (WqT/WkT/W6T as [c_in, c_out]; ci-block tiles) ----
            wsb = {}
            for nm, src in (("wk", wkt_in), ("wq", wqt_in), ("w6", w6t_in)):
                tiles = []
                for ci in range(CB):
                    t = wpool.tile([128, C], f32, tag=f"{nm}{ci}",
                                   name=f"{nm}{ci}")
                    nc.sync.dma_start(out=t[:], in_=src[ci * 128:(ci + 1) * 128, :])
                    if mode == "bf16":
                        tb = wpool.tile([128, C], mmdt, tag=f"{nm}b{ci}",
                                        name=f"{nm}b{ci}")
                        nc.vector.tensor_copy(out=tb[:], in_=t[:])
                        t = tb
                    tiles.append(t)
                wsb[nm] = tiles

            # ---- x (own image, c-major) ----
            x_sb = []
            for ci in range(CB):
                t = xpool.tile([128, HW], f32, tag=f"x{ci}", name=f"x{ci}")
                nc.sync.dma_start(out=t[:], in_=x_in[ci * 128:(ci + 1) * 128, :])
                if mode == "bf16":
                    tb = xpool.tile([128, HW], mmdt, tag=f"xb{ci}", name=f"xb{ci}")
                    nc.vector.tensor_copy(out=tb[:], in_=t[:])
                    t = tb
                x_sb.append(t)

            def linear(wname, bias_t, h, co, out_tile, out_slice, out_dt_is_mm):
                """out[:, out_slice] = (W @ x)[co block, 512-col half h] + bias."""
                ps = ps_m.tile([128, 512], f32, tag="ps_misc", name="ps_lin")
                for ci in range(CB):
                    nc.tensor.matmul(
                        ps[:],
                        mm_ap(wsb[wname][ci][:, co * 128:(co + 1) * 128]),
                        mm_ap(x_sb[ci][:, h * 512:(h + 1) * 512]),
                        start=(ci == 0), stop=(ci == CB - 1))
                nc.scalar.activation(out_tile[:, out_slice], ps[:], AF.Identity,
                                     bias=bias_t[:, co:co + 1], scale=1.0)

            # ---- k local shard -> DRAM bounce -> AllGather (2 key halves) ----
            kg = []
            for h in range(KH):
                kb = dram.tile([C, 512], mmdt, tag=f"kb{h}", name=f"kb{h}")
                for co in range(CB):
                    ke = kepool.tile([128, 512], mmdt, tag="ke", name="ke")
                    linear("wk", bias_sb["bk"], h, co, ke, slice(0, 512), True)
                    nc.sync.dma_start(out=kb[co * 128:(co + 1) * 128, :], in_=ke[:])
                g = dram.tile([NCORES * C, 512], mmdt, tag=f"kg{h}",
                              name=f"kg{h}", addr_space="Shared")
                nc.gpsimd.collective_compute(
                    "AllGather", ALU.bypass,
                    replica_groups=[list(range(NCORES))],
                    ins=[kb[:].opt()], outs=[g[:].opt()])
                kg.append(g)

            # ---- qT (own queries, c-major) ----
            q_sb = []
            for co in range(CB):
                t = qpool.tile([128, HW], mmdt, tag=f"q{co}", name=f"q{co}")
                for h in range(KH):
                    linear("wq", bias_sb["bq"], h, co, t,
                           slice(h * 512, (h + 1) * 512), True)
                q_sb.append(t)

            # ---- y = W6 @ x + b6 (f32; gating applied at the end) ----
            y_sb = []
            for co in range(CB):
                t = qpool.tile([128, HW], f32, tag=f"y{co}", name=f"y{co}")
                for h in range(KH):
                    linear("w6", bias_sb["b6"], h, co, t,
                           slice(h * 512, (h + 1) * 512), False)
                y_sb.append(t)

            # ---- scores + per-image max, streamed per (key-half, image) ----
            # mpart[qb][:, img] = max over key half 0; after half 1 it holds
            # the full per-image max.
            mpart = [redpool.tile([128, NCORES], f32, tag=f"mp{qb}",
                                  name=f"mp{qb}") for qb in range(QB)]
            for h in range(KH):
                for img in range(NCORES):
                    kin = []
                    for ci in range(CB):
                        t = kinpool.tile([128, 512], mmdt, tag=f"kin{ci}",
                                         name=f"kin{ci}")
                        base = img * C + ci * 128
                        nc.sync.dma_start(out=t[:], in_=kg[h][base:base + 128, :])
                        kin.append(t)
                    for qb in range(QB):
                        ps = ps_s.tile([128, 512], f32, tag="ps_s", name="ps_s")
                        for ci in range(CB):
                            nc.tensor.matmul(
                                ps[:],
                                mm_ap(q_sb[ci][:, qb * 128:(qb + 1) * 128]),
                                kin[ci][:] if mode != "f32r" else mm_ap(kin[ci][:]),
                                start=(ci == 0), stop=(ci == CB - 1))
                        if h == 0:
                            nc.vector.tensor_reduce(
                                mpart[qb][:, img:img + 1], ps[:],
                                axis=AX.X, op=ALU.max)
                        else:
                            m1 = redpool.tile([128, 1], f32, tag="m1",
                                              name="m1", bufs=6)
                            nc.vector.tensor_reduce(m1[:], ps[:],
                                                    axis=AX.X, op=ALU.max)
                            nc.vector.tensor_tensor(
                                mpart[qb][:, img:img + 1],
                                mpart[qb][:, img:img + 1], m1[:], op=ALU.max)

            # ---- softmax over the core's 1024 queries ----
            # X8[:, qb] = sum over images of per-image max (mean folded into
            # the exp scale). exp without max-subtraction is safe: |xw*scale|
            # stays O(1) for this distribution.
            X8 = redpool.tile([128, QB], f32, tag="X8", name="X8")
            for qb in range(QB):
                nc.vector.tensor_reduce(X8[:, qb:qb + 1], mpart[qb][:],
                                        axis=AX.X, op=ALU.add)
            EX = redpool.tile([128, QB], f32, tag="EX", name="EX")
            nc.scalar.activation(EX[:], X8[:], AF.Exp, bias=0.0,
                                 scale=SCALE / NCORES)
            # total = sum of EX over all 1024 entries (free dim, then
            # partitions via a ones-vector matmul).
            S1 = redpool.tile([128, 1], f32, tag="S1", name="S1")
            nc.vector.tensor_reduce(S1[:], EX[:], axis=AX.X, op=ALU.add)
            ps_tot = ps_m.tile([128, 512], f32, tag="ps_misc", name="ps_tot")
            nc.tensor.matmul(ps_tot[:1, :1], ones_col[:], S1[:],
                             start=True, stop=True)
            tot = redpool.tile([1, 1], f32, tag="tot", name="tot")
            nc.vector.tensor_copy(out=tot[:], in_=ps_tot[:1, :1])
            rcp = redpool.tile([1, 1], f32, tag="rcp", name="rcp")
            nc.vector.reciprocal(rcp[:], tot[:])
            # broadcast 1/total to 128 partitions, then W8 = EX * (1/total)
            ps_rb = ps_m.tile([128, 512], f32, tag="ps_misc", name="ps_rb")
            nc.tensor.matmul(ps_rb[:, :1], ones_row[:], rcp[:],
                             start=True, stop=True)
            rb = redpool.tile([128, 1], f32, tag="rb", name="rb")
            nc.vector.tensor_copy(out=rb[:], in_=ps_rb[:, :1])
            W8 = redpool.tile([128, QB], f32, tag="W8", name="W8")
            nc.vector.tensor_scalar(W8[:], EX[:], rb[:], None, op0=ALU.mult)

            # ---- gating row: W8 [128 part, 8] -> wrow [1, 1024] ----
            # query index = qb*128 + p; bounce through DRAM and read back
            # with a transposed AP to flatten across partitions.
            wr_d = dram.tile([128, QB], f32, tag="wr_d", name="wr_d")
            nc.sync.dma_start(out=wr_d[:, :], in_=W8[:, :])
            wrow = redpool.tile([1, HW], f32, tag="wrow", name="wrow")
            nc.sync.dma_start(
                out=wrow[0:1, :].rearrange("a (q p) -> a q p", q=QB, p=128),
                in_=wr_d[:, :].transpose([1, 0]))
            # broadcast to all partitions via ones[128,1] @ wrow
            B_sb = redpool.tile([128, HW], f32, tag="B_sb", name="B_sb")
            for h in range(KH):
                ps_b = ps_m.tile([128, 512], f32, tag="ps_misc", name="ps_b")
                nc.tensor.matmul(ps_b[:], ones_row[:],
                                 wrow[0:1, h * 512:(h + 1) * 512],
                                 start=True, stop=True)
                nc.scalar.copy(out=B_sb[:, h * 512:(h + 1) * 512], in_=ps_b[:])

            # ---- out = y * gating ----
            for co in range(CB):
                o = outpool.tile([128, HW], f32, tag="o", name="o")
                nc.vector.tensor_mul(o[:], y_sb[co][:], B_sb[:])
                nc.sync.dma_start(out=out_ext[co * 128:(co + 1) * 128, :],
                                  in_=o[:])

    nc.compile()
    return nc


_BUILT = {}


def _get_nc(mode=MM_MODE):
    if mode not in _BUILT:
        _BUILT[mode] = build_kernel(mode)
    return _BUILT[mode]


def make_in_maps(x, Wq, bq, Wk, bk, W6, b6):
    x = np.asarray(x, dtype=np.float32).reshape(B, C, HW)
    wqt = np.ascontiguousarray(np.asarray(Wq, np.float32).T)
    wkt = np.ascontiguousarray(np.asarray(Wk, np.float32).T)
    w6t = np.ascontiguousarray(np.asarray(W6, np.float32).T)
    bqc = np.ascontiguousarray(np.asarray(bq, np.float32).reshape(C, 1))
    bkc = np.ascontiguousarray(np.asarray(bk, np.float32).reshape(C, 1))
    b6c = np.ascontiguousarray(np.asarray(b6, np.float32).reshape(C, 1))
    return [
        {"x": np.ascontiguousarray(x[b]), "wqt": wqt, "wkt": wkt, "w6t": w6t,
         "bq": bqc, "bk": bkc, "b6": b6c}
        for b in range(B)
    ]


def kernel(x, Wq, bq, Wk, bk, W6, b6, _trace=False):
    from concourse import bass_utils
    nc = _get_nc()
    in_maps = make_in_maps(x, Wq, bq, Wk, bk, W6, b6)
    res = bass_utils.run_bass_kernel_spmd(
        nc, in_maps, core_ids=list(range(NCORES)), trace=_trace)
    out = np.stack([np.asarray(res.results[i]["out"]) for i in range(NCORES)])
    out = out.reshape(B, C, H, W).astype(np.float32)
    if _trace:
        return out, res
    return out
